# revision 57
# baseline (speedup 1.0000x reference)
"""Distributed causal multi-head attention block for Trainium2 (8 NeuronCores).

Problem: B=4, S=2048, E=1024, H=16 heads, fp32.
    q/k/v = Linear(query/key/value); causal softmax attention; out = Linear(attn).

Sharding: DP=4 over batch x TP=2 over heads. Core c = 2*b + g handles batch b
with heads [8g, 8g+8). Per-core kernel structure (single fused Tile graph):
  - K projection prefix (kT in d-major layout), then a software-pipelined
    merged phase: per q-tile, the next q-tile's V/Q projection tiles and
    ready out-projection tiles are interleaved between attention heads so
    TensorE fills the ACT(exp)-bound stretches.
  - Attention is computed in the *transposed* orientation, scoresT[k, q]:
    no max-subtraction (scores are O(1) by construction), no transposes
    anywhere; the softmax denominator comes from an extra ones-column in the
    AV matmul; normalization is a DVE reciprocal + rank-1 broadcast matmul.
    Causal masking: fully-masked blocks are skipped, diagonal blocks are
    column-restricted and masked with a precomputed 0/1 mask multiply.
  - The attn output (attnT, [512, 2048] d-major) is exchanged between core
    pairs with 5 small AllGathers (one per q-tile; the last q-tile is split
    in half) so communication hides under attention compute.
  - out-proj computes this core's 512 *output columns* (host slices Wo per
    core), keeping the instruction graph rank-symmetric (SPMD-safe).

All matmuls run in float32r (tf32, 4x faster than fp32 on the PE): inputs are
pre-rounded to tf32 on the host, PE accumulation is fp32, so the only error is
the tf32 input rounding (~4e-4 relative vs the fp32 reference).
"""
import sys

if "/opt/trn_rl_repo" not in sys.path:
    sys.path.insert(0, "/opt/trn_rl_repo")

import numpy as np

import concourse.bacc as bacc
import concourse.tile as tile
import concourse.mybir as mybir
import concourse.bass_utils as bass_utils

f32 = mybir.dt.float32
f32r = mybir.dt.float32r
bf16 = mybir.dt.bfloat16
Exp = mybir.ActivationFunctionType.Exp

N_CORES = 8
B, S, E = 4, 2048, 1024
H, D = 16, 64
HC = 512            # per-core head dims (8 heads x 64)
SCALE = D ** -0.5
SQ = 512            # q-tile width (columns of scoresT)
SK = 128            # k-chunk (partition rows of scoresT)
NQT = S // SQ       # 4 q-tiles
NE = E // 128       # 8 contraction chunks of the E dim


def tf32_round(x: np.ndarray) -> np.ndarray:
    u = np.ascontiguousarray(x, dtype=np.float32).view(np.uint32)
    u = (u + 0x0FFF + ((u >> 13) & 1)) & np.uint32(0xFFFFE000)
    return u.view(np.float32)


def build_nc(skip_cc=False, lag=2, eager=8, early_op=True):
    nc = bacc.Bacc("TRN2", target_bir_lowering=False, debug=False,
                   num_devices=N_CORES)

    xq = nc.declare_dram_parameter("xq", [E, S], f32, isOutput=False)
    xk = nc.declare_dram_parameter("xk", [E, S], f32, isOutput=False)
    xv = nc.declare_dram_parameter("xv", [E, S], f32, isOutput=False)
    wq = nc.declare_dram_parameter("wq", [E, HC], f32, isOutput=False)
    wk = nc.declare_dram_parameter("wk", [E, HC], f32, isOutput=False)
    wv = nc.declare_dram_parameter("wv", [E, HC], f32, isOutput=False)
    wo = nc.declare_dram_parameter("wo", [E, HC], f32, isOutput=False)
    biases = nc.declare_dram_parameter("biases", [2, 2 * HC], f32, isOutput=False)
    masks = nc.declare_dram_parameter("masks", [128, 4, SQ], bf16, isOutput=False)
    vones = nc.declare_dram_parameter("vones", [128, 16, 8], f32, isOutput=False)
    ones = nc.declare_dram_parameter("ones", [65, SQ], f32, isOutput=False)
    out = nc.declare_dram_parameter("out", [S, HC], f32, isOutput=True)

    # AllGather staging: my attnT [512, 2048] split into four S-quarters so
    # each collective launches as soon as its q-tile finishes.
    agin = [nc.dram_tensor(f"agin{i}", [HC, SQ], f32r) for i in range(3)]
    agout = [nc.dram_tensor(f"agout{i}", [2, HC, SQ], f32r) for i in range(3)]
    # q-tile 3's exchange is split in half so the first part overlaps the
    # last heads' attention and only a 0.5MB collective is exposed at the end
    agin3 = [nc.dram_tensor(f"agin3{i}", [HC // 2, SQ], f32r) for i in range(2)]
    agout3 = [nc.dram_tensor(f"agout3{i}", [2, HC // 2, SQ], f32r)
              for i in range(2)]
    RG = [[0, 1], [2, 3], [4, 5], [6, 7]]

    HS = SQ // 2  # 256-wide half-slabs of the input stream

    with tile.TileContext(nc) as tc:
        with tc.tile_pool(name="persist", bufs=1) as pp, \
             tc.tile_pool(name="xsp", bufs=2) as xsp, \
             tc.tile_pool(name="qtp", bufs=2) as qtp, \
             tc.tile_pool(name="att", bufs=lag + 2) as att, \
             tc.tile_pool(name="attr", bufs=2) as attr, \
             tc.tile_pool(name="op", bufs=2) as op, \
             tc.tile_pool(name="opo", bufs=2) as opo, \
             tc.tile_pool(name="psA", bufs=2, space="PSUM") as psA, \
             tc.tile_pool(name="psS", bufs=lag + 1, space="PSUM") as psS, \
             tc.tile_pool(name="psAV", bufs=2, space="PSUM") as psAV, \
             tc.tile_pool(name="psB", bufs=1, space="PSUM") as psB:
            kT = pp.tile([128, 4, S], f32r)       # [p, m, s]: k-dim = m*128+p
            v4 = pp.tile([128, 16, 8, 65], f32r)  # [p, sc, h, j]: v row sc*128+p
            masks_t = pp.tile([128, 4, SQ], bf16)
            ones_t = pp.tile([65, SQ], f32r)
            bt = pp.tile([65, 2 * HC], f32r)  # p0: bq|bk, p64: bv|bo
            wq_t = pp.tile([128, NE, HC], f32r)
            wv_t = pp.tile([128, NE, HC], f32r)
            # wk lives in its own pool: its slot is handed to wo mid-loop,
            # after the last kT tile is produced
            wkp_cm = tc.tile_pool(name="wkp", bufs=1)
            wkp = wkp_cm.__enter__()
            wk_t = wkp.tile([128, NE, HC], f32r)
            wo_holder = {}

            def dma_w_half(dst, w_dram, i):
                half = NE // 2
                nc.sync.dma_start(
                    out=dst[:, i * half:(i + 1) * half, :],
                    in_=w_dram.ap().rearrange("(c p) n -> p c n", p=128)
                    [:, i * half:(i + 1) * half, :].bitcast(f32r))

            def dma_w(dst, w_dram, eng=None):
                dma_w_half(dst, w_dram, 0)
                dma_w_half(dst, w_dram, 1)

            def load_half(x_dram, n, half):
                xs = xsp.tile([128, NE, HS], f32r, tag="x")
                off = n * SQ + half * HS
                nc.sync.dma_start(
                    out=xs[:],
                    in_=x_dram.ap()
                    .rearrange("(c p) s -> p c s", p=128)[:, :, off:off + HS]
                    .bitcast(f32r))
                return xs

            dma_w_half(wk_t, wk, 0)
            nc.sync.dma_start(out=ones_t[:], in_=ones[:, :].bitcast(f32r))
            nc.sync.dma_start(out=bt[0:1, :], in_=biases[0:1, :].bitcast(f32r))
            nc.sync.dma_start(out=bt[64:65, :], in_=biases[1:2, :].bitcast(f32r))

            def qk_tile(dst_ap_fn, w_t, b_ap, xs, m):
                # one [128, HS] output tile of a q/k-style projection
                ps = psA.tile([128, HS], f32, tag="pp")
                for kc in range(NE):
                    nc.tensor.matmul(ps[:], w_t[:, kc, m * 128:(m + 1) * 128],
                                     xs[:, kc, :], start=(kc == 0), stop=False)
                nc.tensor.matmul(ps[:], b_ap[:, m * 128:(m + 1) * 128],
                                 ones_t[0:1, 0:HS], start=False, stop=True)
                nc.vector.tensor_copy(dst_ap_fn(), ps[:])

            def v_tile(xs, sc, mm):
                # one [128 S-rows, 512 v-dims] tile of the V projection
                ps = psA.tile([128, HC], f32, tag="pp")
                for kc in range(NE):
                    nc.tensor.matmul(ps[:], xs[:, kc, mm * 128:(mm + 1) * 128],
                                     wv_t[:, kc, :], start=(kc == 0), stop=False)
                nc.tensor.matmul(ps[:], ones_t[64:65, 0:128], bt[64:65, 0:HC],
                                 start=False, stop=True)
                nc.vector.tensor_copy(
                    v4[:, sc, :, 0:64],
                    ps[:].rearrange("p (h j) -> p h j", h=8))

            qtiles = [None] * NQT
            xk_cur = [None]
            xv_cur = [None]
            xq_cur = [None]

            def proj_tasks(n):
                # kT-slab n + v-slab n + q-slab n as resumable tile tasks
                qtiles[n] = qtp.tile([128, 4, SQ], f32r, tag="qt",
                                     name=f"qtile{n}")
                tasks = []
                for half in range(2):
                    for mm in range(2):
                        tasks.append(("v", n, mm, half))
                    for m in range(4):
                        tasks.append(("q", n, m, half))
                return tasks

            def run_task(t):
                kind, n, m, half = t
                if kind == "v":
                    if m == 0:
                        xv_cur[0] = load_half(xv, n, half)
                    v_tile(xv_cur[0], n * 4 + half * 2 + m, m)
                elif kind == "q":
                    if m == 0:
                        xq_cur[0] = load_half(xq, n, half)
                    qtl = qtiles[n]
                    qk_tile(lambda: qtl[:, m, half * HS:(half + 1) * HS],
                            wq_t, bt[0:1, 0:HC], xq_cur[0], m)
                else:
                    outproj_tile(n, m)

            def outproj_tile(part, mm):
                wo_t = wo_holder["wo_t"]
                lt = op.tile([128, 2, 4, 128], f32r, tag="lt")
                sl = slice(mm * 128, (mm + 1) * 128)
                if part == 3:
                    if skip_cc:
                        for i in range(2):
                            src = agin3[i].ap().rearrange(
                                "(ic p) s -> p ic s", p=128)[:, :, sl]
                            nc.sync.dma_start(out=lt[:, 0, 2 * i:2 * i + 2, :],
                                              in_=src)
                            nc.sync.dma_start(out=lt[:, 1, 2 * i:2 * i + 2, :],
                                              in_=src)
                    else:
                        for i in range(2):
                            for j in range(2):
                                nc.sync.dma_start(
                                    out=lt[:, j, 2 * i:2 * i + 2, :],
                                    in_=agout3[i].ap().rearrange(
                                        "j (ic p) s -> p j ic s",
                                        p=128)[:, j, :, sl])
                elif skip_cc:
                    src = agin[part].ap().rearrange(
                        "(ic p) s -> p ic s", p=128)[:, :, sl]
                    nc.sync.dma_start(out=lt[:, 0, :, :], in_=src)
                    nc.sync.dma_start(out=lt[:, 1, :, :], in_=src)
                else:
                    nc.sync.dma_start(
                        out=lt[:],
                        in_=agout[part].ap().rearrange(
                            "j (ic p) s -> p j ic s", p=128)[:, :, :, sl])
                po_ = psA.tile([128, HC], f32, tag="pp")
                for kcg in range(NE):
                    nc.tensor.matmul(po_[:], lt[:, kcg // 4, kcg % 4, :],
                                     wo_t[:, kcg, :],
                                     start=(kcg == 0), stop=False)
                nc.tensor.matmul(po_[:], ones_t[64:65, 0:128],
                                 bt[64:65, HC:2 * HC], start=False, stop=True)
                ot = opo.tile([128, HC], f32, tag="ot")
                nc.vector.tensor_copy(ot[:], po_[:])
                nc.sync.dma_start(
                    out=out[part * SQ + mm * 128:part * SQ + (mm + 1) * 128, :],
                    in_=ot[:])

            # ---------------- prefix: full K projection ----------------
            for n in range(4):
                for half in range(2):
                    xs = load_half(xk, n, half)
                    if n == 0 and half == 0:
                        # second wk half + small loads ride behind the first
                        # k half-slab
                        dma_w_half(wk_t, wk, 1)
                        nc.sync.dma_start(out=masks_t[:], in_=masks[:, :, :])
                        nc.sync.dma_start(out=v4[:, :, :, 64],
                                          in_=vones[:, :, :].bitcast(f32r))
                    for m in range(4):
                        off = n * SQ + half * HS
                        qk_tile(lambda m=m, off=off: kT[:, m, off:off + HS],
                                wk_t, bt[0:1, HC:2 * HC], xs, m)
            dma_w(wv_t, wv)
            dma_w(wq_t, wq)
            # wk's SBUF slot is handed to wo_t; attention pools open here
            wkp_cm.__exit__(None, None, None)
            wop_cm = tc.tile_pool(name="wop", bufs=1)
            wop = wop_cm.__enter__()
            wo_t = wop.tile([128, NE, HC], f32r)
            wo_holder["wo_t"] = wo_t
            wo_holder["cm"] = wop_cm

            # ---------------- merged v/q projections + attention ----------
            for t in proj_tasks(0):
                run_task(t)
            dma_w(wo_t, wo, eng=nc.sync)

            work = []
            pending_fin = None
            for qt in range(NQT):
                if qt + 1 < NQT:
                    work.extend(proj_tasks(qt + 1))
                if qt == 2:
                    work.extend(("op", 0, mm, 0) for mm in range(4))
                if qt == 3:
                    work.extend(("op", part, mm, 0)
                                for part in (1, 2) for mm in range(4))
                for h in range(8):
                    m, po = h // 2, 64 * (h % 2)
                    pav = psAV.tile([65, SQ], f32, tag="av")
                    nkc = (qt + 1) * (SQ // SK)
                    pts = {}
                    qtl = qtiles[qt]

                    def issue_score(kc, qt=qt, m=m, po=po, pts=pts, qtl=qtl):
                        r = kc - 4 * qt
                        # diagonal blocks: columns < r*128 are fully masked;
                        # restrict the score matmul too while its free dim
                        # stays >=256 (f32r full-rate threshold)
                        s0 = r * SK if r in (1, 2) else 0
                        pscore = psS.tile([128, SQ], f32, tag="sc")
                        nc.tensor.matmul(
                            pscore[:, s0:],
                            kT[po:po + 64, m, kc * SK:(kc + 1) * SK],
                            qtl[po:po + 64, m, s0:],
                            start=True, stop=True)
                        pt = att.tile([128, SQ], f32r, tag="pt")
                        if 0 <= r <= 3:
                            # columns < r*128 are fully masked: skip them
                            c0 = r * SK
                            praw = attr.tile([128, SQ], f32, tag="praw")
                            nc.scalar.activation(praw[:, c0:], pscore[:, c0:],
                                                 Exp, scale=SCALE)
                            nc.vector.tensor_tensor(
                                pt[:, c0:], praw[:, c0:], masks_t[:, r, c0:],
                                op=mybir.AluOpType.mult)
                            pts[kc] = (pt, c0)
                        else:
                            nc.scalar.activation(pt[:], pscore[:], Exp,
                                                 scale=SCALE)
                            pts[kc] = (pt, 0)

                    for kc in range(min(lag, nkc)):
                        issue_score(kc)
                    # finalize the previous head while this head's score
                    # pipeline fills, so TensorE never waits on the DVE
                    # reciprocal chain
                    if pending_fin is not None:
                        pending_fin()
                        pending_fin = None
                    if qt == 3 and h == 4 and not skip_cc:
                        nc.gpsimd.collective_compute(
                            "AllGather", mybir.AluOpType.bypass,
                            replica_groups=RG,
                            ins=[agin3[0].ap().opt()],
                            outs=[agout3[0].ap().opt()])
                    for kc in range(nkc):
                        if kc + lag < nkc:
                            issue_score(kc + lag)
                        pt, c0 = pts.pop(kc)
                        # kc==0 always has c0==0, so start covers the whole
                        # [65, 512] accumulator
                        nc.tensor.matmul(pav[:, c0:], v4[:, kc, h, :],
                                         pt[:, c0:],
                                         start=(kc == 0), stop=(kc == nkc - 1))

                    def finalize(qt=qt, h=h, pav=pav):
                        rt = attr.tile([1, SQ], f32r, tag="rt")
                        with nc.allow_low_precision(reason="tf32 pipeline"):
                            nc.vector.reciprocal(rt[:], pav[64:65, :])
                        pb = psB.tile([64, SQ], f32, tag="bc")
                        nc.tensor.matmul(pb[:], ones_t[0:1, 0:64], rt[:],
                                         start=True, stop=True)
                        pbs = attr.tile([64, SQ], f32, tag="pbs")
                        nc.vector.tensor_copy(pbs[:], pb[:])
                        at = attr.tile([64, SQ], f32r, tag="at")
                        nc.vector.tensor_tensor(at[:], pav[0:64, :], pbs[:],
                                                op=mybir.AluOpType.mult)
                        if qt == 3:
                            dst = agin3[h // 4][(h % 4) * 64:(h % 4 + 1) * 64, :]
                        else:
                            dst = agin[qt][h * 64:(h + 1) * 64, :]
                        nc.sync.dma_start(out=dst, in_=at[:])

                    pending_fin = finalize
                    npop = -(-len(work) // max(1, eager - h)) if work else 0
                    for _ in range(min(npop, len(work))):
                        run_task(work.pop(0))
                if pending_fin is not None:
                    pending_fin()
                    pending_fin = None
                while work:
                    run_task(work.pop(0))
                if not skip_cc:
                    src_ag = agin[qt].ap() if qt < 3 else agin3[1].ap()
                    dst_ag = agout[qt].ap() if qt < 3 else agout3[1].ap()
                    nc.gpsimd.collective_compute(
                        "AllGather", mybir.AluOpType.bypass,
                        replica_groups=RG,
                        ins=[src_ag.opt()], outs=[dst_ag.opt()])
            for mm in range(4):
                outproj_tile(3, mm)
            wo_holder["cm"].__exit__(None, None, None)

    nc.compile()
    return nc


_NC_CACHE = None


def _get_nc():
    global _NC_CACHE
    if _NC_CACHE is None:
        _NC_CACHE = build_nc()
    return _NC_CACHE


def _prepare_in_maps(query, key, value, Wq, bq, Wk, bk, Wv, bv, Wo, bo):
    query = np.asarray(query, dtype=np.float32)
    key = np.asarray(key, dtype=np.float32)
    value = np.asarray(value, dtype=np.float32)

    xqT = [tf32_round(np.ascontiguousarray(query[b].T)) for b in range(B)]
    xkT = [tf32_round(np.ascontiguousarray(key[b].T)) for b in range(B)]
    xvT = [tf32_round(np.ascontiguousarray(value[b].T)) for b in range(B)]

    wq_g, wk_g, wv_g, wo_g = [], [], [], []
    biases_g = []
    for g in range(2):
        sl = slice(g * HC, (g + 1) * HC)
        wq_g.append(tf32_round(np.ascontiguousarray(np.asarray(Wq)[sl, :].T)))
        wk_g.append(tf32_round(np.ascontiguousarray(np.asarray(Wk)[sl, :].T)))
        wv_g.append(tf32_round(np.ascontiguousarray(np.asarray(Wv)[sl, :].T)))
        wo_g.append(tf32_round(np.ascontiguousarray(np.asarray(Wo)[sl, :].T)))
        biases_g.append(tf32_round(np.stack([
            np.concatenate([np.asarray(bq)[sl], np.asarray(bk)[sl]]),
            np.concatenate([np.asarray(bv)[sl], np.asarray(bo)[sl]])])))

    import ml_dtypes
    p = np.arange(128)[:, None, None]
    r = np.arange(4)[None, :, None]
    qn = np.arange(SQ)[None, None, :]
    masks = ((p + r * 128) <= qn).astype(ml_dtypes.bfloat16)
    vones = np.ones((128, 16, 8), dtype=np.float32)
    ones = np.ones((65, SQ), dtype=np.float32)

    in_maps = []
    for c in range(N_CORES):
        b, g = c // 2, c % 2
        in_maps.append({
            "xq": xqT[b], "xk": xkT[b], "xv": xvT[b],
            "wq": wq_g[g], "wk": wk_g[g], "wv": wv_g[g], "wo": wo_g[g],
            "biases": biases_g[g],
            "masks": masks, "vones": vones, "ones": ones,
        })
    return in_maps


def run(trace=False, **inputs):
    in_maps = _prepare_in_maps(**inputs)
    nc = _get_nc()
    res = bass_utils.run_bass_kernel_spmd(
        nc, in_maps, core_ids=list(range(N_CORES)), trace=trace)
    full = np.empty((B, S, E), dtype=np.float32)
    for c in range(N_CORES):
        b, g = c // 2, c % 2
        full[b, :, g * HC:(g + 1) * HC] = res.results[c]["out"]
    return full, res


def kernel(**inputs) -> np.ndarray:
    full, _ = run(trace=False, **inputs)
    return full


def bench(n_iters=5, repeats=5, nc=None, **inputs):
    """Estimate on-device NEFF time: chain n_iters executions with a tiny
    data dependency (no CSE, strict serialization), time with device-resident
    inputs, and report the marginal per-iteration wall time."""
    import time
    import jax
    from jax.sharding import Mesh, PartitionSpec
    from jax.experimental.shard_map import shard_map
    import concourse.bass2jax as bass2jax
    import concourse.mybir as mb

    if nc is None:
        nc = _get_nc()
    in_maps = _prepare_in_maps(**inputs)
    bass2jax.install_neuronx_cc_hook()

    partition_name = nc.partition_id_tensor.name if nc.partition_id_tensor else None
    in_names, out_names, out_avals = [], [], []
    for alloc in nc.m.functions[0].allocations:
        if not isinstance(alloc, mb.MemoryLocationSet):
            continue
        name = alloc.memorylocations[0].name
        if alloc.kind == "ExternalInput":
            if name != partition_name:
                in_names.append(name)
        elif alloc.kind == "ExternalOutput":
            out_names.append(name)
            out_avals.append(
                jax.core.ShapedArray(tuple(alloc.tensor_shape),
                                     mb.dt.np(alloc.dtype)))
    n_params = len(in_names)
    all_in_names = list(in_names) + list(out_names)
    if partition_name is not None:
        all_in_names.append(partition_name)
    ones_idx = in_names.index("ones")

    def _body(*args):
        operands = list(args)
        if partition_name is not None:
            operands.append(bass2jax.partition_id_tensor())
        outs = bass2jax._bass_exec_p.bind(
            *operands,
            out_avals=tuple(out_avals),
            in_names=tuple(all_in_names),
            out_names=tuple(out_names),
            lowering_input_output_aliases=(),
            sim_require_finite=True,
            sim_require_nnan=True,
            nc=nc)
        return tuple(outs)

    devices = jax.devices()[:N_CORES]
    mesh = Mesh(np.asarray(devices), ("core",))
    n_outs = len(out_names)
    in_specs = (PartitionSpec("core"),) * (n_params + n_outs)
    out_specs = (PartitionSpec("core"),) * n_outs

    per_core = [[np.asarray(m[name]) for name in in_names] for m in in_maps]
    concat_in = [np.concatenate([per_core[c][i] for c in range(N_CORES)], axis=0)
                 for i in range(n_params)]
    concat_zeros = [np.zeros((N_CORES * a.shape[0], *a.shape[1:]), a.dtype)
                    for a in out_avals]

    sharding = jax.sharding.NamedSharding(mesh, PartitionSpec("core"))
    dev_in = [jax.device_put(x, sharding) for x in concat_in + concat_zeros]

    # donate the output buffers and chain each call's outputs into the next
    # call's donated outputs: executions serialize on-device, memory stays
    # bounded, and M iterations aggregate enough device time to dominate the
    # ~100ms axon RTT quantum.
    donate = tuple(range(n_params, n_params + n_outs))
    fn = jax.jit(shard_map(_body, mesh=mesh, in_specs=in_specs,
                           out_specs=out_specs, check_rep=False),
                 keep_unused=True, donate_argnums=donate)
    params = dev_in[:n_params]
    outs = tuple(dev_in[n_params:])
    outs = fn(*params, *outs)  # warm
    jax.block_until_ready(outs)

    def run_m(m):
        nonlocal outs
        t0 = time.perf_counter()
        for _ in range(m):
            outs = fn(*params, *outs)
        jax.block_until_ready(outs)
        return time.perf_counter() - t0

    m_lo, m_hi = 8, 8 + n_iters
    t_lo = min(run_m(m_lo) for _ in range(repeats))
    t_hi = min(run_m(m_hi) for _ in range(repeats))
    marginal = (t_hi - t_lo) / (m_hi - m_lo)
    return marginal * 1e9, {"m_lo": (m_lo, t_lo), "m_hi": (m_hi, t_hi)}


_BASE_NC = None


def _bench_baseline(mesh):
    import time
    import jax
    from jax.sharding import PartitionSpec
    from jax.experimental.shard_map import shard_map
    import concourse.bass2jax as bass2jax

    global _BASE_NC
    if _BASE_NC is None:
        nc = bacc.Bacc("TRN2", target_bir_lowering=False, debug=False,
                       num_devices=N_CORES)
        one = nc.declare_dram_parameter("one", [1, SQ], f32, isOutput=False)
        outp = nc.declare_dram_parameter("out", [1, SQ], f32, isOutput=True)
        with tile.TileContext(nc) as tc:
            with tc.tile_pool(name="sb", bufs=1) as sb:
                t = sb.tile([1, SQ], f32)
                nc.sync.dma_start(out=t[:], in_=one[:, :])
                nc.sync.dma_start(out=outp[:, :], in_=t[:])
        nc.compile()
        _BASE_NC = nc
    nc = _BASE_NC

    partition_name = nc.partition_id_tensor.name if nc.partition_id_tensor else None
    in_names = ["one", "out"]
    if partition_name is not None:
        in_names.append(partition_name)
    out_avals = (jax.core.ShapedArray((1, SQ), np.float32),)

    def _body(*args):
        operands = list(args)
        if partition_name is not None:
            operands.append(bass2jax.partition_id_tensor())
        outs = bass2jax._bass_exec_p.bind(
            *operands, out_avals=out_avals, in_names=tuple(in_names),
            out_names=("out",), lowering_input_output_aliases=(),
            sim_require_finite=True, sim_require_nnan=True, nc=nc)
        return tuple(outs)

    sharding = jax.sharding.NamedSharding(mesh, PartitionSpec("core"))
    ones = jax.device_put(np.ones((N_CORES, SQ), np.float32), sharding)
    zeros = jax.device_put(np.zeros((N_CORES, SQ), np.float32), sharding)
    fn = jax.jit(shard_map(_body, mesh=mesh,
                           in_specs=(PartitionSpec("core"),) * 2,
                           out_specs=(PartitionSpec("core"),),
                           check_rep=False), keep_unused=True)
    jax.block_until_ready(fn(ones, zeros))
    best = float("inf")
    for _ in range(20):
        t0 = time.perf_counter()
        jax.block_until_ready(fn(ones, zeros))
        best = min(best, time.perf_counter() - t0)
    return best
